# revision 20
# baseline (speedup 1.0000x reference)
"""GATv2Conv layer on 8 Trainium2 NeuronCores (Bass/Tile).

Strategy (edge-parallel, dst-sorted, zero cross-core collectives):
  - Host sorts edges by dst and partitions nodes into 8 contiguous ranges of
    6250; each core owns all edges targeting its node range (~100k edges).
  - Within a core, dst nodes are tiled 128 at a time (49 tiles); each tile's
    edges are padded to 18 chunks of 128 edges (max real count is 2174).
  - Per 128-edge chunk, everything is PE matmuls in feature-major layout:
      s^T[f,e]   = We^T @ eaT  +  xr^T @ Onehot[n,e]  +  (xl[src])^T
      logits^T   = lrelu(s)^T-slice as lhsT  @  att_blockdiag   -> [e, 4]
      scatter    = Onehot[e,n]^T @ (ex * xl_g | ex)  accumulated in PSUM
  - Softmax skips the segment-max (|logit| <= ~8 for this data, exp is safe
    in fp32); denominator is scattered alongside the messages, and the
    division happens once per node.
  - The xl table (x @ Wl, no bias) is built redundantly by every core,
    kept feature-major in SBUF, and per-edge columns are pulled with the
    gpsimd ap_gather compute instruction (all 8 Q7 cores; no DMA rings, no
    xbar).  ap_gather requires 4-byte elements, so the bf16 table is viewed
    as f32 *pairs* of adjacent node columns (idx = src >> 1) and each tile's
    edge slots are grouped by src parity (chunks 0-8 even, 9-17 odd); the
    consumer APs read the gathered pairs with stride 2 and base offset equal
    to the group parity.
    bl is algebraically moved: s gets it via the lrelu bias (per-feature,
    partition axis), and the aggregation gets it post-division (bl is zero
    in practice so that add is skipped).
"""

import sys

import numpy as np

sys.path.insert(0, "/opt/trn_rl_repo")

N, E, D, H, C, EDGE_DIM = 50000, 800000, 128, 4, 32, 16
NEG_SLOPE = 0.2
LN_EPS = 1e-5
N_CORES = 8
NPC = N // N_CORES            # 6250 nodes per core
TILES = (NPC + 127) // 128    # 49 dst tiles per core
NPAD = TILES * 128            # 6272
CHUNKS = 18                   # 128-edge chunks per tile (max observed 17)
TE = CHUNKS * 128             # 2304 padded edges per tile
NT = ((N // 128) + 1) * 128   # xl table cols padded: 50048
NE2 = NT // 2                 # f32-pair table cols: 25024
GROUPS = [(0, 4), (4, 8), (8, 12), (12, 16), (16, 18)]
CH_A = 9                      # chunks in the even-src parity group
NIA = CH_A * 128              # 1152 slots per parity group
ICA = TE // 16                # 144 int16 index columns per tile

TRACE = False                 # set by test.py to capture a HW profile
import os as _os
def _envint(n):
    v = _os.environ.get(n)
    return None if v is None else int(v)
BISECT_TILES = _envint("BISECT_TILES")
BISECT_SLABS = _envint("BISECT_SLABS")
SKIP_GATHER = _os.environ.get("SKIP_GATHER") == "1"
LAST_EXEC_TIME_NS = None
LAST_RESULTS = None

_CACHE = {}


def _np_dt(mdt):
    from concourse import mybir
    return np.dtype(mybir.dt.np(mdt))


def _xlg_ap(bass, xlgb, s0, n, par):
    """Strided bf16 AP over the gathered f32-pair buffer: slot j's xl column
    lives at bf16 col 2*j + par for n consecutive slots starting at s0."""
    base = xlgb[:, 2 * s0 + par : 2 * (s0 + n)]
    return bass.AP(base.tensor, base.offset, [list(base.ap[0]), [2, n]])


def _build_nc(flags):
    """flags = (add_post, use_gamma, use_beta)."""
    import concourse.bacc as bacc
    import concourse.bass as bass
    import concourse.tile as tile
    from concourse import mybir

    add_post, use_gamma, use_beta = flags
    f32, bf16, i32 = mybir.dt.float32, mybir.dt.bfloat16, mybir.dt.int32
    fp8 = mybir.dt.float8e4
    i16 = mybir.dt.int16
    AF = mybir.ActivationFunctionType
    OP = mybir.AluOpType

    nc = bacc.Bacc(None, target_bir_lowering=False)

    # --- shared (same array for all cores) inputs -------------------------
    xT_d = nc.dram_tensor("xT", [128, NT], bf16, kind="ExternalInput")
    wl_d = nc.dram_tensor("wl", [128, 128], bf16, kind="ExternalInput")
    wr_d = nc.dram_tensor("wr", [128, 128], bf16, kind="ExternalInput")
    we_d = nc.dram_tensor("we", [16, 128], bf16, kind="ExternalInput")
    attb_d = nc.dram_tensor("attb", [128, 4], bf16, kind="ExternalInput")
    ident_d = nc.dram_tensor("ident", [128, 128], bf16, kind="ExternalInput")
    lrb_d = nc.dram_tensor("lrb", [128, 1], f32, kind="ExternalInput")
    if add_post:
        post_d = nc.dram_tensor("post", [128, 128], f32, kind="ExternalInput")
    if use_gamma:
        gam_d = nc.dram_tensor("gam", [128, 128], f32, kind="ExternalInput")
    if use_beta:
        bet_d = nc.dram_tensor("bet", [128, 128], f32, kind="ExternalInput")

    # --- per-core inputs ---------------------------------------------------
    idx_d = nc.dram_tensor("idx", [TILES * 128, ICA], i16, kind="ExternalInput")
    oh_d = nc.dram_tensor("oh", [TILES * 128, 2 * TE], fp8, kind="ExternalInput")
    eaT_d = nc.dram_tensor("eaT", [TILES * 16, TE], bf16, kind="ExternalInput")
    xoT_d = nc.dram_tensor("xoT", [128, NPAD], bf16, kind="ExternalInput")
    # residual x in blocked layout: xob[p, t*128+f] = x[t*128+p, f]
    xob_d = nc.dram_tensor("xob", [128, NPAD], f32, kind="ExternalInput")
    out_d = nc.dram_tensor("out", [128, NPAD], f32, kind="ExternalOutput")

    with tile.TileContext(nc) as tc:
        with (
            tc.tile_pool(name="tabp", bufs=1) as tabp,
            tc.tile_pool(name="constp", bufs=1) as constp,
        ):
            # feature-major xl table: table_sb[f, n] = xl[n, f]
            table_sb = tabp.tile([128, NT], bf16)
            table_f32 = table_sb[:, :].bitcast(f32)  # [128, NE2] pairs

            wl_sb = constp.tile([128, 128], bf16)
            nc.sync.dma_start(wl_sb, wl_d[:])
            wr_sb = constp.tile([128, 128], bf16)
            nc.sync.dma_start(wr_sb, wr_d[:])
            we_sb = constp.tile([16, 128], bf16)
            nc.sync.dma_start(we_sb, we_d[:])
            attb_sb = constp.tile([128, 4], bf16)
            nc.sync.dma_start(attb_sb, attb_d[:])
            ident_sb = constp.tile([128, 128], bf16)
            nc.sync.dma_start(ident_sb, ident_d[:])
            lrb_sb = constp.tile([128, 1], f32)
            nc.sync.dma_start(lrb_sb, lrb_d[:])
            zero_sb = constp.tile([128, 1], f32)
            nc.vector.memset(zero_sb, 0.0)
            eps_sb = constp.tile([128, 1], f32)
            nc.vector.memset(eps_sb, LN_EPS)
            if add_post:
                post_sb = constp.tile([128, 128], f32)
                nc.sync.dma_start(post_sb, post_d[:])
            if use_gamma:
                gam_sb = constp.tile([128, 128], f32)
                nc.sync.dma_start(gam_sb, gam_d[:])
            if use_beta:
                bet_sb = constp.tile([128, 128], f32)
                nc.sync.dma_start(bet_sb, bet_d[:])

            # ---------------- phase A: xl table = (x @ Wl)^T (bf16) -------
            # wl stays the stationary operand; x^T slabs stream through at
            # N=512, producing feature-major xl^T directly.
            with (
                tc.tile_pool(name="xslabp", bufs=3) as xslabp,
                tc.tile_pool(name="psAp", bufs=4, space="PSUM") as psAp,
            ):
                SLAB = 2048
                _slabs = list(range(0, NT, SLAB))
                if BISECT_SLABS is not None:
                    _slabs = _slabs[:BISECT_SLABS]
                for s in _slabs:
                    w = min(SLAB, NT - s)
                    xslab = xslabp.tile([128, SLAB], bf16)
                    nc.sync.dma_start(xslab[:, :w], xT_d[:, s : s + w])
                    for k in range(0, w, 512):
                        kw = min(512, w - k)
                        psA = psAp.tile([128, 512], f32, space="PSUM")
                        nc.tensor.matmul(
                            out=psA[:, :kw],
                            lhsT=wl_sb,
                            rhs=xslab[:, k : k + kw],
                            start=True,
                            stop=True,
                        )
                        nc.any.tensor_copy(
                            table_sb[:, s + k : s + k + kw], psA[:, :kw]
                        )

            # ---------------- phase B: edge tiles ------------------------
            with (
                tc.tile_pool(name="idxp", bufs=4) as idxp,
                tc.tile_pool(name="ohp", bufs=3) as ohp,
                tc.tile_pool(name="eatp", bufs=2) as eatp,
                tc.tile_pool(name="xotp", bufs=3) as xotp,
                tc.tile_pool(name="xlgp", bufs=4) as xlgp,
                tc.tile_pool(name="xrp", bufs=2) as xrp,
                tc.tile_pool(name="lrsp", bufs=3) as lrsp,
                tc.tile_pool(name="msgp", bufs=3) as msgp,
                tc.tile_pool(name="stagep", bufs=1) as stagep,
                tc.tile_pool(name="slabp", bufs=1) as slabp,
                tc.tile_pool(name="psSp", bufs=2, space="PSUM") as psSp,
                tc.tile_pool(name="psLp", bufs=1, space="PSUM") as psLp,
                tc.tile_pool(name="psOp", bufs=2, space="PSUM") as psOp,
                tc.tile_pool(name="psXp", bufs=1, space="PSUM") as psXp,
                tc.tile_pool(name="psGp", bufs=2, space="PSUM") as psGp,
            ):
                ntiles = TILES if BISECT_TILES is None else BISECT_TILES
                # per-node scatter results staged for the batched LN phase
                stage = stagep.tile([128, TILES, 132], bf16)
                def _emit_gather(u):
                    idx_sb = idxp.tile([128, ICA], i16)
                    nc.sync.dma_start(idx_sb, idx_d[u * 128 : (u + 1) * 128, :])
                    xlg2 = xlgp.tile([128, TE], f32)
                    if SKIP_GATHER:
                        nc.vector.memset(xlg2[:, :], 0.0)
                    else:
                        nc.gpsimd.ap_gather(
                            out_ap=xlg2[:, :],
                            in_ap=table_f32,
                            idxs_ap=idx_sb[:, :],
                            channels=128,
                            num_elems=NE2,
                            d=1,
                            num_idxs=TE,
                        )
                    return xlg2

                LOOKAHEAD = 2
                xlg_ring = {}
                for u in range(min(LOOKAHEAD, ntiles)):
                    xlg_ring[u] = _emit_gather(u)
                for t in range(ntiles):
                    if t + LOOKAHEAD < ntiles:
                        xlg_ring[t + LOOKAHEAD] = _emit_gather(t + LOOKAHEAD)
                    oh_sb = ohp.tile([128, 2 * TE], fp8)
                    nc.sync.dma_start(oh_sb, oh_d[t * 128 : (t + 1) * 128, :])
                    eaT_sb = eatp.tile([16, TE], bf16)
                    nc.sync.dma_start(eaT_sb, eaT_d[t * 16 : (t + 1) * 16, :])

                    # xr tile for own 128 nodes (no bias; folded into lrelu)
                    xoT_sb = xotp.tile([128, 128], bf16)
                    nc.sync.dma_start(xoT_sb, xoT_d[:, t * 128 : (t + 1) * 128])
                    psX = psXp.tile([128, 128], f32, space="PSUM")
                    nc.tensor.matmul(
                        out=psX,
                        lhsT=xoT_sb,
                        rhs=wr_sb,
                        start=True,
                        stop=True,
                    )
                    xr_sb = xrp.tile([128, 128], bf16)
                    nc.any.tensor_copy(xr_sb, psX)

                    # gathered xl pair-columns for this tile (issued ahead)
                    xlg2 = xlg_ring.pop(t)
                    xlgb = xlg2[:, :].bitcast(bf16)  # [128, 2*TE]

                    # stage 1: s^T accumulation + lrelu, per chunk-group
                    lrsT_sb = lrsp.tile([128, TE], bf16)
                    msg_sb = msgp.tile([128, CHUNKS, 132], bf16)
                    for g0, g1 in GROUPS:
                        ng = g1 - g0
                        gw = ng * 128
                        psS = psSp.tile([128, 512], f32, space="PSUM", tag="psS")
                        nc.tensor.matmul(
                            out=psS[:, :gw], lhsT=we_sb,
                            rhs=eaT_sb[:, g0 * 128 : g1 * 128],
                            start=True, stop=False,
                        )
                        nc.tensor.matmul(
                            out=psS[:, :gw], lhsT=xr_sb,
                            rhs=oh_sb[:, TE + g0 * 128 : TE + g1 * 128],
                            start=False, stop=False,
                        )
                        # gathered-xl contribution, split at the parity edge
                        parts = []
                        if g0 < CH_A:
                            parts.append((g0, min(g1, CH_A), 0))
                        if g1 > CH_A:
                            parts.append((max(g0, CH_A), g1, 1))
                        for a, b, par in parts:
                            nc.tensor.matmul(
                                out=psS[:, (a - g0) * 128 : (b - g0) * 128],
                                lhsT=ident_sb,
                                rhs=_xlg_ap(bass, xlgb, a * 128, (b - a) * 128, par),
                                start=False,
                                stop=True,
                            )
                        nc.scalar.activation(
                            lrsT_sb[:, g0 * 128 : g1 * 128],
                            psS[:, :gw],
                            AF.Prelu,
                            bias=lrb_sb[:, :],
                            scale=1.0,
                            alpha=NEG_SLOPE,
                        )

                    # stage 2: logits for all chunks, then one exp straight
                    # into the msg denominator lanes
                    psL = psLp.tile([128, CHUNKS, 4], f32, space="PSUM", tag="psL")
                    for ch in range(CHUNKS):
                        nc.tensor.matmul(
                            out=psL[:, ch, :],
                            lhsT=lrsT_sb[:, ch * 128 : (ch + 1) * 128],
                            rhs=attb_sb,
                            start=True,
                            stop=True,
                        )
                    nc.scalar.activation(
                        msg_sb[:, :, 128:132], psL[:, :, :], AF.Exp,
                        bias=zero_sb[:, :],
                    )

                    # stage 3: transpose gathered columns to edge-major and
                    # weight them by ex
                    for g0, g1 in GROUPS:
                        ng = g1 - g0
                        psG = psGp.tile([128, 512], f32, space="PSUM", tag="psG")
                        for i, ch in enumerate(range(g0, g1)):
                            par = 0 if ch < CH_A else 1
                            nc.tensor.matmul(
                                out=psG[:, i * 128 : (i + 1) * 128],
                                lhsT=_xlg_ap(bass, xlgb, ch * 128, 128, par),
                                rhs=ident_sb,
                                start=True,
                                stop=True,
                            )
                        ex_bcast = msg_sb[:, g0:g1, 128:132].broadcast_to(
                            [128, ng, 4, 32]
                        )
                        psG3 = bass.AP(
                            psG.tensor, psG.offset,
                            [list(psG.ap[0]), [128, ng], [1, 128]],
                        )
                        nc.vector.tensor_tensor(
                            out=msg_sb[:, g0:g1, 0:128],
                            in0=psG3,
                            in1=ex_bcast,
                            op=OP.mult,
                        )

                    # scatter-sum into [node, 128 msg + 4 denom]
                    psO = psOp.tile([128, 132], f32, space="PSUM", tag="psO")
                    for ch in range(CHUNKS):
                        nc.tensor.matmul(
                            out=psO,
                            lhsT=oh_sb[:, ch * 128 : (ch + 1) * 128],
                            rhs=msg_sb[:, ch, :],
                            start=(ch == 0),
                            stop=(ch == CHUNKS - 1),
                        )
                    nc.any.tensor_copy(stage[:, t, :], psO)

                # ---- batched node phase over tile slabs: normalize, gelu,
                # residual, layernorm, all in blocked [p, t, f] layout
                ST = 4
                X = mybir.AxisListType.X
                for t0 in range(0, ntiles, ST):
                    S = min(ST, ntiles - t0)
                    sl = slice(t0, t0 + S)
                    den = slabp.tile([128, ST, 4], f32, tag="den")
                    nc.vector.tensor_scalar_add(
                        den[:, :S, :], stage[:, sl, 128:132], 1e-16
                    )
                    rden = slabp.tile([128, ST, 4], f32, tag="rden")
                    nc.vector.reciprocal(rden[:, :S, :], den[:, :S, :])
                    h_sb = slabp.tile([128, ST, 128], f32, tag="h")
                    nc.vector.tensor_tensor(
                        out=h_sb[:, :S, :].rearrange("p s (h c) -> p s h c", h=4),
                        in0=stage[:, sl, 0:128].rearrange(
                            "p s (h c) -> p s h c", h=4
                        ),
                        in1=rden[:, :S, :].broadcast_to([128, S, 4, 32]),
                        op=OP.mult,
                    )
                    if add_post:
                        post3 = bass.AP(
                            post_sb.tensor, post_sb.offset,
                            [list(post_sb.ap[0]), [0, S], [1, 128]],
                        )
                        nc.vector.tensor_add(h_sb[:, :S, :], h_sb[:, :S, :], post3)
                    g_sb = slabp.tile([128, ST, 128], f32, tag="g")
                    nc.scalar.activation(
                        g_sb[:, :S, :], h_sb[:, :S, :], AF.Gelu, bias=zero_sb[:, :]
                    )
                    xo_sb = slabp.tile([128, ST, 128], f32, tag="xo")
                    nc.sync.dma_start(
                        xo_sb[:, :S, :].rearrange("p s f -> p (s f)"),
                        xob_d[:, t0 * 128 : (t0 + S) * 128],
                    )
                    nc.vector.tensor_add(g_sb[:, :S, :], g_sb[:, :S, :], xo_sb[:, :S, :])
                    musum = slabp.tile([128, ST], f32, tag="musum")
                    nc.vector.tensor_reduce(
                        musum[:, :S], g_sb[:, :S, :], X, OP.add
                    )
                    mu = slabp.tile([128, ST], f32, tag="mu")
                    nc.vector.tensor_scalar_mul(mu[:, :S], musum[:, :S], 1.0 / 128.0)
                    c_sb = slabp.tile([128, ST, 128], f32, tag="h")
                    nc.vector.tensor_tensor(
                        out=c_sb[:, :S, :],
                        in0=g_sb[:, :S, :],
                        in1=mu[:, :S].broadcast_to([128, S, 128]),
                        op=OP.subtract,
                    )
                    c2_sb = slabp.tile([128, ST, 128], f32, tag="g")
                    nc.vector.tensor_tensor(
                        out=c2_sb[:, :S, :], in0=c_sb[:, :S, :], in1=c_sb[:, :S, :],
                        op=OP.mult,
                    )
                    varsum = slabp.tile([128, ST], f32, tag="varsum")
                    nc.vector.tensor_reduce(
                        varsum[:, :S], c2_sb[:, :S, :], X, OP.add
                    )
                    std = slabp.tile([128, ST], f32, tag="std")
                    nc.scalar.activation(
                        std[:, :S], varsum[:, :S], AF.Sqrt,
                        bias=eps_sb[:, :], scale=1.0 / 128.0,
                    )
                    rstd = slabp.tile([128, ST], f32, tag="rstd")
                    nc.vector.reciprocal(rstd[:, :S], std[:, :S])
                    o_sb = slabp.tile([128, ST, 128], f32, tag="g")
                    nc.vector.tensor_tensor(
                        out=o_sb[:, :S, :],
                        in0=c_sb[:, :S, :],
                        in1=rstd[:, :S].broadcast_to([128, S, 128]),
                        op=OP.mult,
                    )
                    if use_gamma:
                        gam3 = bass.AP(
                            gam_sb.tensor, gam_sb.offset,
                            [list(gam_sb.ap[0]), [0, S], [1, 128]],
                        )
                        nc.vector.tensor_mul(o_sb[:, :S, :], o_sb[:, :S, :], gam3)
                    if use_beta:
                        bet3 = bass.AP(
                            bet_sb.tensor, bet_sb.offset,
                            [list(bet_sb.ap[0]), [0, S], [1, 128]],
                        )
                        nc.vector.tensor_add(o_sb[:, :S, :], o_sb[:, :S, :], bet3)
                    nc.sync.dma_start(
                        out_d[:, t0 * 128 : (t0 + S) * 128],
                        o_sb[:, :S, :].rearrange("p s f -> p (s f)"),
                    )

    return nc


def _host_prep(x, edge_index, edge_attr, Wl, bl, Wr, br, We, att, bias, gamma, beta):
    import ml_dtypes

    bf16 = ml_dtypes.bfloat16
    fp8 = ml_dtypes.float8_e4m3

    src = np.ascontiguousarray(edge_index[0]).astype(np.int64)
    dst = np.ascontiguousarray(edge_index[1]).astype(np.int64)
    order = np.argsort(dst, kind="stable")
    ssrc = src[order].astype(np.int32)
    sdst = dst[order].astype(np.int32)
    sea = np.ascontiguousarray(edge_attr, dtype=np.float32)[order]

    xTp = np.zeros((128, NT), dtype=bf16)
    xTp[:, :N] = x.T.astype(bf16)

    shared = {
        "xT": xTp,
        "wl": Wl.astype(bf16),
        "wr": Wr.astype(bf16),
        "we": We.astype(bf16),
        "ident": np.eye(128, dtype=bf16),
        "lrb": (bl + br).astype(np.float32).reshape(128, 1),
    }
    attb = np.zeros((128, 4), dtype=np.float32)
    for h in range(H):
        attb[h * C : (h + 1) * C, h] = att[h]
    shared["attb"] = attb.astype(bf16)

    post = bl + bias  # added after the denom division, before gelu
    add_post = bool(np.any(post != 0.0))
    use_gamma = bool(np.any(gamma != 1.0))
    use_beta = bool(np.any(beta != 0.0))

    if add_post:
        shared["post"] = np.broadcast_to(post.astype(np.float32), (128, 128)).copy()
    if use_gamma:
        shared["gam"] = np.broadcast_to(gamma.astype(np.float32), (128, 128)).copy()
    if use_beta:
        shared["bet"] = np.broadcast_to(beta.astype(np.float32), (128, 128)).copy()

    in_maps = []
    for c in range(N_CORES):
        n0 = c * NPC
        e0 = np.searchsorted(sdst, n0)
        e1 = np.searchsorted(sdst, n0 + NPC)
        csrc = ssrc[e0:e1]
        cdst = sdst[e0:e1] - n0
        cea = sea[e0:e1]

        idx = np.zeros((TILES, 128, ICA), dtype=np.int16)
        eaT = np.zeros((TILES, 16, TE), dtype=bf16)
        oh = np.zeros((TILES, 128, 2, CHUNKS, 128), dtype=fp8)  # [t,p,(en|ne),ch,n]
        ohen = oh[:, :, 0].reshape(TILES, 128, CHUNKS, 128)
        ohne = oh[:, :, 1].reshape(TILES, 128, TE)
        tb = np.searchsorted(cdst, np.arange(0, NPAD + 1, 128))
        one8 = np.ones((), dtype=fp8)
        prow = np.arange(128) % 16
        for t in range(TILES):
            sl = slice(tb[t], tb[t + 1])
            es = csrc[sl]
            ed = (cdst[sl] - t * 128).astype(np.int64)
            ea_t = cea[sl]
            # split slots by src parity; idx addresses f32 pairs (src >> 1)
            idlist = np.zeros(TE, np.int16)
            mA = (es & 1) == 0
            for hi, msk in enumerate((mA, ~mA)):
                k = int(msk.sum())
                assert k <= NIA, f"core {c} tile {t} parity {hi}: {k} > {NIA}"
                j = np.arange(k) + hi * NIA  # slot within the tile
                idlist[j] = (es[msk] >> 1).astype(np.int16)
                eaT[t, :, j] = ea_t[msk].astype(bf16)
                ohen[t, j % 128, j // 128, ed[msk]] = one8
                ohne[t, ed[msk], j] = one8
            idx[t] = idlist.reshape(ICA, 16).T[prow]

        xoT = np.zeros((128, NPAD), dtype=bf16)
        xoT[:, :NPC] = x[n0 : n0 + NPC].T.astype(bf16)
        xown = np.zeros((NPAD, 128), dtype=np.float32)
        xown[:NPC] = x[n0 : n0 + NPC]
        # blocked residual layout: xob[p, t*128+f] = x[n0 + t*128 + p, f]
        xob = np.ascontiguousarray(
            xown.reshape(TILES, 128, 128).transpose(1, 0, 2).reshape(128, NPAD)
        )

        m = dict(shared)
        m.update(
            idx=idx.reshape(TILES * 128, ICA),
            eaT=eaT.reshape(TILES * 16, TE),
            oh=oh.reshape(TILES * 128, 2 * TE),
            xoT=xoT,
            xob=xob,
        )
        in_maps.append(m)
    return in_maps, (add_post, use_gamma, use_beta)


def kernel(x, edge_index, edge_attr, Wl, bl, Wr, br, We, att, bias, gamma, beta):
    global LAST_EXEC_TIME_NS, LAST_RESULTS
    x = np.asarray(x, np.float32)
    edge_index = np.asarray(edge_index)
    edge_attr = np.asarray(edge_attr, np.float32)
    Wl = np.asarray(Wl, np.float32)
    bl = np.asarray(bl, np.float32)
    Wr = np.asarray(Wr, np.float32)
    br = np.asarray(br, np.float32)
    We = np.asarray(We, np.float32)
    att = np.asarray(att, np.float32)
    bias = np.asarray(bias, np.float32)
    gamma = np.asarray(gamma, np.float32)
    beta = np.asarray(beta, np.float32)

    in_maps, flags = _host_prep(
        x, edge_index, edge_attr, Wl, bl, Wr, br, We, att, bias, gamma, beta
    )

    if flags not in _CACHE:
        nc = _build_nc(flags)
        nc.finalize()
        _CACHE[flags] = nc
    nc = _CACHE[flags]

    from concourse.bass_utils import run_bass_kernel_spmd

    res = run_bass_kernel_spmd(
        nc, in_maps, list(range(N_CORES)), trace=TRACE
    )
    LAST_RESULTS = res
    LAST_EXEC_TIME_NS = res.exec_time_ns
    out = np.concatenate(
        [
            res.results[c]["out"]
            .reshape(128, TILES, 128)
            .transpose(1, 0, 2)
            .reshape(NPAD, 128)[:NPC]
            for c in range(N_CORES)
        ],
        axis=0,
    )
    return out.astype(np.float32)


# revision 21
# speedup vs baseline: 3.0560x; 3.0560x over previous
"""GATv2Conv layer on 8 Trainium2 NeuronCores (Bass/Tile).

Strategy (edge-parallel, dst-sorted, zero cross-core collectives):
  - Host sorts edges by dst and partitions nodes into 8 contiguous ranges of
    6250; each core owns all edges targeting its node range (~100k edges).
  - Within a core, dst nodes are tiled 128 at a time (49 tiles); each tile's
    edges are padded to 18 chunks of 128 edges (max real count is 2174).
  - Per 128-edge chunk, everything is PE matmuls in feature-major layout:
      s^T[f,e]   = We^T @ eaT  +  xr^T @ Onehot[n,e]  +  (xl[src])^T
      logits^T   = lrelu(s)^T-slice as lhsT  @  att_blockdiag   -> [e, 4]
      scatter    = Onehot[e,n]^T @ (ex * xl_g | ex)  accumulated in PSUM
  - Softmax skips the segment-max (|logit| <= ~8 for this data, exp is safe
    in fp32); denominator is scattered alongside the messages, and the
    division happens once per node.
  - The xl table (x @ Wl, no bias) is built redundantly by every core,
    kept feature-major in SBUF, and per-edge columns are pulled with the
    gpsimd ap_gather compute instruction (all 8 Q7 cores; no DMA rings, no
    xbar).  ap_gather requires 4-byte elements, so the bf16 table is viewed
    as f32 *pairs* of adjacent node columns (idx = src >> 1) and each tile's
    edge slots are grouped by src parity (chunks 0-8 even, 9-17 odd); the
    consumer APs read the gathered pairs with stride 2 and base offset equal
    to the group parity.
    bl is algebraically moved: s gets it via the lrelu bias (per-feature,
    partition axis), and the aggregation gets it post-division (bl is zero
    in practice so that add is skipped).
"""

import sys

import numpy as np

sys.path.insert(0, "/opt/trn_rl_repo")

N, E, D, H, C, EDGE_DIM = 50000, 800000, 128, 4, 32, 16
NEG_SLOPE = 0.2
LN_EPS = 1e-5
N_CORES = 8
NPC = N // N_CORES            # 6250 nodes per core
TILES = (NPC + 127) // 128    # 49 dst tiles per core
NPAD = TILES * 128            # 6272
CHUNKS = 18                   # 128-edge chunks per tile (max observed 17)
TE = CHUNKS * 128             # 2304 padded edges per tile
NT = ((N // 128) + 1) * 128   # xl table cols padded: 50048
GROUPS = [(0, 4), (4, 8), (8, 12), (12, 16), (16, 18)]
CH_A = 9                      # chunks gathered from table half A (src < HALF)
NIA = CH_A * 128              # 1152 slots per half
ICOLS = NIA // 16             # 72 int16 index columns per half
HALF = 25088                  # xl-table split row (multiple of 128)
ICA = 2 * ICOLS               # 144 int16 index columns per tile

TRACE = False                 # set by test.py to capture a HW profile
import os as _os
def _envint(n):
    v = _os.environ.get(n)
    return None if v is None else int(v)
BISECT_TILES = _envint("BISECT_TILES")
BISECT_SLABS = _envint("BISECT_SLABS")
SKIP_GATHER = _os.environ.get("SKIP_GATHER") == "1"
LAST_EXEC_TIME_NS = None
LAST_RESULTS = None

_CACHE = {}


def _np_dt(mdt):
    from concourse import mybir
    return np.dtype(mybir.dt.np(mdt))


def _emit_dma_gathers(nc, bass, table_sb, xlgT_sb, idx_sb):
    """Two SBUF-source transposed gathers: xlgT_sb[:, h*NIA + e] = xl[src_e]^T."""
    for h in range(2):
        base = xlgT_sb[:, h * NIA : (h + 1) * NIA]
        out3 = bass.AP(
            base.tensor, base.offset, [list(base.ap[0]), [NIA, 1], [1, NIA]]
        )
        src_ap = (
            table_sb[:, 0 : HALF // 128 * 128]
            if h == 0
            else table_sb[:, HALF // 128 * 128 : NT // 128 * 128]
        )
        nc.gpsimd.dma_gather(
            out_ap=out3,
            in_ap=src_ap,
            idxs_ap=idx_sb[:, h * ICOLS : (h + 1) * ICOLS],
            num_idxs=NIA,
            num_idxs_reg=NIA,
            elem_size=128,
            transpose=True,
            single_packet=False,
            sbuf_tokens_per_rank=128,
            sbuf_free_dim_per_rank=256,
            sbuf_byte_offset=0,
        )


def _build_nc(flags):
    """flags = (add_post, use_gamma, use_beta)."""
    import concourse.bacc as bacc
    import concourse.bass as bass
    import concourse.tile as tile
    from concourse import mybir

    add_post, use_gamma, use_beta = flags
    f32, bf16, i32 = mybir.dt.float32, mybir.dt.bfloat16, mybir.dt.int32
    fp8 = mybir.dt.float8e4
    i16 = mybir.dt.int16
    AF = mybir.ActivationFunctionType
    OP = mybir.AluOpType

    nc = bacc.Bacc(None, target_bir_lowering=False)

    # --- shared (same array for all cores) inputs -------------------------
    xT_d = nc.dram_tensor("xT", [128, NT], bf16, kind="ExternalInput")
    wl_d = nc.dram_tensor("wl", [128, 128], bf16, kind="ExternalInput")
    wr_d = nc.dram_tensor("wr", [128, 128], bf16, kind="ExternalInput")
    we_d = nc.dram_tensor("we", [16, 128], bf16, kind="ExternalInput")
    attb_d = nc.dram_tensor("attb", [128, 4], bf16, kind="ExternalInput")
    ident_d = nc.dram_tensor("ident", [128, 128], bf16, kind="ExternalInput")
    lrb_d = nc.dram_tensor("lrb", [128, 1], f32, kind="ExternalInput")
    if add_post:
        post_d = nc.dram_tensor("post", [128, 128], f32, kind="ExternalInput")
    if use_gamma:
        gam_d = nc.dram_tensor("gam", [128, 128], f32, kind="ExternalInput")
    if use_beta:
        bet_d = nc.dram_tensor("bet", [128, 128], f32, kind="ExternalInput")

    # --- per-core inputs ---------------------------------------------------
    idx_d = nc.dram_tensor("idx", [TILES * 128, ICA], i16, kind="ExternalInput")
    oh_d = nc.dram_tensor("oh", [TILES * 128, 2 * TE], fp8, kind="ExternalInput")
    eaT_d = nc.dram_tensor("eaT", [TILES * 16, TE], bf16, kind="ExternalInput")
    xoT_d = nc.dram_tensor("xoT", [128, NPAD], bf16, kind="ExternalInput")
    # residual x in blocked layout: xob[p, t*128+f] = x[t*128+p, f]
    xob_d = nc.dram_tensor("xob", [128, NPAD], f32, kind="ExternalInput")
    out_d = nc.dram_tensor("out", [128, NPAD], f32, kind="ExternalOutput")

    with tile.TileContext(nc) as tc:
        with (
            tc.tile_pool(name="tabp", bufs=1) as tabp,
            tc.tile_pool(name="constp", bufs=1) as constp,
        ):
            # blocked node-major xl table: 128-col stripe k holds xl rows
            # [128k, 128k+128) with partition = row-within-stripe
            table_sb = tabp.tile([128, NT], bf16)

            wl_sb = constp.tile([128, 128], bf16)
            nc.sync.dma_start(wl_sb, wl_d[:])
            wr_sb = constp.tile([128, 128], bf16)
            nc.sync.dma_start(wr_sb, wr_d[:])
            we_sb = constp.tile([16, 128], bf16)
            nc.sync.dma_start(we_sb, we_d[:])
            attb_sb = constp.tile([128, 4], bf16)
            nc.sync.dma_start(attb_sb, attb_d[:])
            ident_sb = constp.tile([128, 128], bf16)
            nc.sync.dma_start(ident_sb, ident_d[:])
            lrb_sb = constp.tile([128, 1], f32)
            nc.sync.dma_start(lrb_sb, lrb_d[:])
            zero_sb = constp.tile([128, 1], f32)
            nc.vector.memset(zero_sb, 0.0)
            eps_sb = constp.tile([128, 1], f32)
            nc.vector.memset(eps_sb, LN_EPS)
            if add_post:
                post_sb = constp.tile([128, 128], f32)
                nc.sync.dma_start(post_sb, post_d[:])
            if use_gamma:
                gam_sb = constp.tile([128, 128], f32)
                nc.sync.dma_start(gam_sb, gam_d[:])
            if use_beta:
                bet_sb = constp.tile([128, 128], f32)
                nc.sync.dma_start(bet_sb, bet_d[:])

            # ---------------- phase A: xl table = x @ Wl (bf16, blocked) --
            with (
                tc.tile_pool(name="xslabp", bufs=3) as xslabp,
                tc.tile_pool(name="psAp", bufs=4, space="PSUM") as psAp,
            ):
                SLAB = 2048
                _slabs = list(range(0, NT, SLAB))
                if BISECT_SLABS is not None:
                    _slabs = _slabs[:BISECT_SLABS]
                for s in _slabs:
                    w = min(SLAB, NT - s)
                    xslab = xslabp.tile([128, SLAB], bf16)
                    nc.sync.dma_start(xslab[:, :w], xT_d[:, s : s + w])
                    for k in range(w // 128):
                        psA = psAp.tile([128, 128], f32, space="PSUM")
                        nc.tensor.matmul(
                            out=psA,
                            lhsT=xslab[:, k * 128 : (k + 1) * 128],
                            rhs=wl_sb,
                            start=True,
                            stop=True,
                        )
                        nc.any.tensor_copy(
                            table_sb[:, s + k * 128 : s + (k + 1) * 128], psA
                        )

            # ---------------- phase B: edge tiles ------------------------
            with (
                tc.tile_pool(name="idxp", bufs=4) as idxp,
                tc.tile_pool(name="ohp", bufs=3) as ohp,
                tc.tile_pool(name="eatp", bufs=2) as eatp,
                tc.tile_pool(name="xotp", bufs=3) as xotp,
                tc.tile_pool(name="xlgp", bufs=4) as xlgp,
                tc.tile_pool(name="xrp", bufs=2) as xrp,
                tc.tile_pool(name="lrsp", bufs=3) as lrsp,
                tc.tile_pool(name="msgp", bufs=3) as msgp,
                tc.tile_pool(name="stagep", bufs=1) as stagep,
                tc.tile_pool(name="slabp", bufs=1) as slabp,
                tc.tile_pool(name="psSp", bufs=2, space="PSUM") as psSp,
                tc.tile_pool(name="psLp", bufs=1, space="PSUM") as psLp,
                tc.tile_pool(name="psOp", bufs=2, space="PSUM") as psOp,
                tc.tile_pool(name="psXp", bufs=1, space="PSUM") as psXp,
                tc.tile_pool(name="psGp", bufs=2, space="PSUM") as psGp,
            ):
                ntiles = TILES if BISECT_TILES is None else BISECT_TILES
                # per-node scatter results staged for the batched LN phase
                stage = stagep.tile([128, TILES, 132], bf16)
                def _emit_gather(u):
                    idx_sb = idxp.tile([128, ICA], i16)
                    nc.sync.dma_start(idx_sb, idx_d[u * 128 : (u + 1) * 128, :])
                    xlgT = xlgp.tile([128, TE], bf16)
                    if SKIP_GATHER:
                        nc.vector.memset(xlgT[:, :], 0.0)
                    else:
                        _emit_dma_gathers(nc, bass, table_sb, xlgT, idx_sb)
                    return xlgT

                LOOKAHEAD = 2
                xlg_ring = {}
                for u in range(min(LOOKAHEAD, ntiles)):
                    xlg_ring[u] = _emit_gather(u)
                for t in range(ntiles):
                    if t + LOOKAHEAD < ntiles:
                        xlg_ring[t + LOOKAHEAD] = _emit_gather(t + LOOKAHEAD)
                    oh_sb = ohp.tile([128, 2 * TE], fp8)
                    nc.sync.dma_start(oh_sb, oh_d[t * 128 : (t + 1) * 128, :])
                    eaT_sb = eatp.tile([16, TE], bf16)
                    nc.sync.dma_start(eaT_sb, eaT_d[t * 16 : (t + 1) * 16, :])

                    # xr tile for own 128 nodes (no bias; folded into lrelu)
                    xoT_sb = xotp.tile([128, 128], bf16)
                    nc.sync.dma_start(xoT_sb, xoT_d[:, t * 128 : (t + 1) * 128])
                    psX = psXp.tile([128, 128], f32, space="PSUM")
                    nc.tensor.matmul(
                        out=psX,
                        lhsT=xoT_sb,
                        rhs=wr_sb,
                        start=True,
                        stop=True,
                    )
                    xr_sb = xrp.tile([128, 128], bf16)
                    nc.any.tensor_copy(xr_sb, psX)

                    # gathered xl columns for this tile (issued ahead)
                    xlgT = xlg_ring.pop(t)

                    # stage 1: s^T accumulation + lrelu, per chunk-group
                    lrsT_sb = lrsp.tile([128, TE], bf16)
                    msg_sb = msgp.tile([128, CHUNKS, 132], bf16)
                    for g0, g1 in GROUPS:
                        ng = g1 - g0
                        gw = ng * 128
                        psS = psSp.tile([128, 512], f32, space="PSUM", tag="psS")
                        nc.tensor.matmul(
                            out=psS[:, :gw], lhsT=we_sb,
                            rhs=eaT_sb[:, g0 * 128 : g1 * 128],
                            start=True, stop=False,
                        )
                        nc.tensor.matmul(
                            out=psS[:, :gw], lhsT=xr_sb,
                            rhs=oh_sb[:, TE + g0 * 128 : TE + g1 * 128],
                            start=False, stop=False,
                        )
                        nc.tensor.matmul(
                            out=psS[:, :gw],
                            lhsT=ident_sb,
                            rhs=xlgT[:, g0 * 128 : g1 * 128],
                            start=False,
                            stop=True,
                        )
                        nc.scalar.activation(
                            lrsT_sb[:, g0 * 128 : g1 * 128],
                            psS[:, :gw],
                            AF.Prelu,
                            bias=lrb_sb[:, :],
                            scale=1.0,
                            alpha=NEG_SLOPE,
                        )

                    # stage 2: logits for all chunks, then one exp straight
                    # into the msg denominator lanes
                    psL = psLp.tile([128, CHUNKS, 4], f32, space="PSUM", tag="psL")
                    for ch in range(CHUNKS):
                        nc.tensor.matmul(
                            out=psL[:, ch, :],
                            lhsT=lrsT_sb[:, ch * 128 : (ch + 1) * 128],
                            rhs=attb_sb,
                            start=True,
                            stop=True,
                        )
                    nc.scalar.activation(
                        msg_sb[:, :, 128:132], psL[:, :, :], AF.Exp,
                        bias=zero_sb[:, :],
                    )

                    # stage 3: transpose gathered columns to edge-major and
                    # weight them by ex
                    for g0, g1 in GROUPS:
                        ng = g1 - g0
                        psG = psGp.tile([128, 512], f32, space="PSUM", tag="psG")
                        for i, ch in enumerate(range(g0, g1)):
                            nc.tensor.matmul(
                                out=psG[:, i * 128 : (i + 1) * 128],
                                lhsT=xlgT[:, ch * 128 : (ch + 1) * 128],
                                rhs=ident_sb,
                                start=True,
                                stop=True,
                            )
                        ex_bcast = msg_sb[:, g0:g1, 128:132].broadcast_to(
                            [128, ng, 4, 32]
                        )
                        psG3 = bass.AP(
                            psG.tensor, psG.offset,
                            [list(psG.ap[0]), [128, ng], [1, 128]],
                        )
                        nc.vector.tensor_tensor(
                            out=msg_sb[:, g0:g1, 0:128],
                            in0=psG3,
                            in1=ex_bcast,
                            op=OP.mult,
                        )

                    # scatter-sum into [node, 128 msg + 4 denom]
                    psO = psOp.tile([128, 132], f32, space="PSUM", tag="psO")
                    for ch in range(CHUNKS):
                        nc.tensor.matmul(
                            out=psO,
                            lhsT=oh_sb[:, ch * 128 : (ch + 1) * 128],
                            rhs=msg_sb[:, ch, :],
                            start=(ch == 0),
                            stop=(ch == CHUNKS - 1),
                        )
                    nc.any.tensor_copy(stage[:, t, :], psO)

                # ---- batched node phase over tile slabs: normalize, gelu,
                # residual, layernorm, all in blocked [p, t, f] layout
                ST = 4
                X = mybir.AxisListType.X
                for t0 in range(0, ntiles, ST):
                    S = min(ST, ntiles - t0)
                    sl = slice(t0, t0 + S)
                    den = slabp.tile([128, ST, 4], f32, tag="den")
                    nc.vector.tensor_scalar_add(
                        den[:, :S, :], stage[:, sl, 128:132], 1e-16
                    )
                    rden = slabp.tile([128, ST, 4], f32, tag="rden")
                    nc.vector.reciprocal(rden[:, :S, :], den[:, :S, :])
                    h_sb = slabp.tile([128, ST, 128], f32, tag="h")
                    nc.vector.tensor_tensor(
                        out=h_sb[:, :S, :].rearrange("p s (h c) -> p s h c", h=4),
                        in0=stage[:, sl, 0:128].rearrange(
                            "p s (h c) -> p s h c", h=4
                        ),
                        in1=rden[:, :S, :].broadcast_to([128, S, 4, 32]),
                        op=OP.mult,
                    )
                    if add_post:
                        post3 = bass.AP(
                            post_sb.tensor, post_sb.offset,
                            [list(post_sb.ap[0]), [0, S], [1, 128]],
                        )
                        nc.vector.tensor_add(h_sb[:, :S, :], h_sb[:, :S, :], post3)
                    g_sb = slabp.tile([128, ST, 128], f32, tag="g")
                    nc.scalar.activation(
                        g_sb[:, :S, :], h_sb[:, :S, :], AF.Gelu, bias=zero_sb[:, :]
                    )
                    xo_sb = slabp.tile([128, ST, 128], f32, tag="xo")
                    nc.sync.dma_start(
                        xo_sb[:, :S, :].rearrange("p s f -> p (s f)"),
                        xob_d[:, t0 * 128 : (t0 + S) * 128],
                    )
                    nc.vector.tensor_add(g_sb[:, :S, :], g_sb[:, :S, :], xo_sb[:, :S, :])
                    musum = slabp.tile([128, ST], f32, tag="musum")
                    nc.vector.tensor_reduce(
                        musum[:, :S], g_sb[:, :S, :], X, OP.add
                    )
                    mu = slabp.tile([128, ST], f32, tag="mu")
                    nc.vector.tensor_scalar_mul(mu[:, :S], musum[:, :S], 1.0 / 128.0)
                    c_sb = slabp.tile([128, ST, 128], f32, tag="h")
                    nc.vector.tensor_tensor(
                        out=c_sb[:, :S, :],
                        in0=g_sb[:, :S, :],
                        in1=mu[:, :S].broadcast_to([128, S, 128]),
                        op=OP.subtract,
                    )
                    c2_sb = slabp.tile([128, ST, 128], f32, tag="g")
                    nc.vector.tensor_tensor(
                        out=c2_sb[:, :S, :], in0=c_sb[:, :S, :], in1=c_sb[:, :S, :],
                        op=OP.mult,
                    )
                    varsum = slabp.tile([128, ST], f32, tag="varsum")
                    nc.vector.tensor_reduce(
                        varsum[:, :S], c2_sb[:, :S, :], X, OP.add
                    )
                    std = slabp.tile([128, ST], f32, tag="std")
                    nc.scalar.activation(
                        std[:, :S], varsum[:, :S], AF.Sqrt,
                        bias=eps_sb[:, :], scale=1.0 / 128.0,
                    )
                    rstd = slabp.tile([128, ST], f32, tag="rstd")
                    nc.vector.reciprocal(rstd[:, :S], std[:, :S])
                    o_sb = slabp.tile([128, ST, 128], f32, tag="g")
                    nc.vector.tensor_tensor(
                        out=o_sb[:, :S, :],
                        in0=c_sb[:, :S, :],
                        in1=rstd[:, :S].broadcast_to([128, S, 128]),
                        op=OP.mult,
                    )
                    if use_gamma:
                        gam3 = bass.AP(
                            gam_sb.tensor, gam_sb.offset,
                            [list(gam_sb.ap[0]), [0, S], [1, 128]],
                        )
                        nc.vector.tensor_mul(o_sb[:, :S, :], o_sb[:, :S, :], gam3)
                    if use_beta:
                        bet3 = bass.AP(
                            bet_sb.tensor, bet_sb.offset,
                            [list(bet_sb.ap[0]), [0, S], [1, 128]],
                        )
                        nc.vector.tensor_add(o_sb[:, :S, :], o_sb[:, :S, :], bet3)
                    nc.sync.dma_start(
                        out_d[:, t0 * 128 : (t0 + S) * 128],
                        o_sb[:, :S, :].rearrange("p s f -> p (s f)"),
                    )

    return nc


def _host_prep(x, edge_index, edge_attr, Wl, bl, Wr, br, We, att, bias, gamma, beta):
    import ml_dtypes

    bf16 = ml_dtypes.bfloat16
    fp8 = ml_dtypes.float8_e4m3

    src = np.ascontiguousarray(edge_index[0]).astype(np.int64)
    dst = np.ascontiguousarray(edge_index[1]).astype(np.int64)
    order = np.argsort(dst, kind="stable")
    ssrc = src[order].astype(np.int32)
    sdst = dst[order].astype(np.int32)
    sea = np.ascontiguousarray(edge_attr, dtype=np.float32)[order]

    xTp = np.zeros((128, NT), dtype=bf16)
    xTp[:, :N] = x.T.astype(bf16)

    shared = {
        "xT": xTp,
        "wl": Wl.astype(bf16),
        "wr": Wr.astype(bf16),
        "we": We.astype(bf16),
        "ident": np.eye(128, dtype=bf16),
        "lrb": (bl + br).astype(np.float32).reshape(128, 1),
    }
    attb = np.zeros((128, 4), dtype=np.float32)
    for h in range(H):
        attb[h * C : (h + 1) * C, h] = att[h]
    shared["attb"] = attb.astype(bf16)

    post = bl + bias  # added after the denom division, before gelu
    add_post = bool(np.any(post != 0.0))
    use_gamma = bool(np.any(gamma != 1.0))
    use_beta = bool(np.any(beta != 0.0))

    if add_post:
        shared["post"] = np.broadcast_to(post.astype(np.float32), (128, 128)).copy()
    if use_gamma:
        shared["gam"] = np.broadcast_to(gamma.astype(np.float32), (128, 128)).copy()
    if use_beta:
        shared["bet"] = np.broadcast_to(beta.astype(np.float32), (128, 128)).copy()

    in_maps = []
    for c in range(N_CORES):
        n0 = c * NPC
        e0 = np.searchsorted(sdst, n0)
        e1 = np.searchsorted(sdst, n0 + NPC)
        csrc = ssrc[e0:e1]
        cdst = sdst[e0:e1] - n0
        cea = sea[e0:e1]

        idx = np.zeros((TILES, 128, ICA), dtype=np.int16)
        eaT = np.zeros((TILES, 16, TE), dtype=bf16)
        oh = np.zeros((TILES, 128, 2, CHUNKS, 128), dtype=fp8)  # [t,p,(en|ne),ch,n]
        ohen = oh[:, :, 0].reshape(TILES, 128, CHUNKS, 128)
        ohne = oh[:, :, 1].reshape(TILES, 128, TE)
        tb = np.searchsorted(cdst, np.arange(0, NPAD + 1, 128))
        one8 = np.ones((), dtype=fp8)
        prow = np.arange(128) % 16
        for t in range(TILES):
            sl = slice(tb[t], tb[t + 1])
            es = csrc[sl]
            ed = (cdst[sl] - t * 128).astype(np.int64)
            ea_t = cea[sl]
            # split slots by src table half; local idx fits int16
            idlist = np.zeros(TE, np.int16)
            mA = es < HALF
            for hi, (msk, base) in enumerate(((mA, 0), (~mA, HALF))):
                k = int(msk.sum())
                assert k <= NIA, f"core {c} tile {t} half {hi}: {k} > {NIA}"
                j = np.arange(k) + hi * NIA  # slot within the tile
                idlist[j] = (es[msk] - base).astype(np.int16)
                eaT[t, :, j] = ea_t[msk].astype(bf16)
                ohen[t, j % 128, j // 128, ed[msk]] = one8
                ohne[t, ed[msk], j] = one8
            # each half wrapped separately into its 72 int16 columns
            for hi in range(2):
                idx[t, :, hi * ICOLS : (hi + 1) * ICOLS] = (
                    idlist[hi * NIA : (hi + 1) * NIA].reshape(ICOLS, 16).T[prow]
                )

        xoT = np.zeros((128, NPAD), dtype=bf16)
        xoT[:, :NPC] = x[n0 : n0 + NPC].T.astype(bf16)
        xown = np.zeros((NPAD, 128), dtype=np.float32)
        xown[:NPC] = x[n0 : n0 + NPC]
        # blocked residual layout: xob[p, t*128+f] = x[n0 + t*128 + p, f]
        xob = np.ascontiguousarray(
            xown.reshape(TILES, 128, 128).transpose(1, 0, 2).reshape(128, NPAD)
        )

        m = dict(shared)
        m.update(
            idx=idx.reshape(TILES * 128, ICA),
            eaT=eaT.reshape(TILES * 16, TE),
            oh=oh.reshape(TILES * 128, 2 * TE),
            xoT=xoT,
            xob=xob,
        )
        in_maps.append(m)
    return in_maps, (add_post, use_gamma, use_beta)


def kernel(x, edge_index, edge_attr, Wl, bl, Wr, br, We, att, bias, gamma, beta):
    global LAST_EXEC_TIME_NS, LAST_RESULTS
    x = np.asarray(x, np.float32)
    edge_index = np.asarray(edge_index)
    edge_attr = np.asarray(edge_attr, np.float32)
    Wl = np.asarray(Wl, np.float32)
    bl = np.asarray(bl, np.float32)
    Wr = np.asarray(Wr, np.float32)
    br = np.asarray(br, np.float32)
    We = np.asarray(We, np.float32)
    att = np.asarray(att, np.float32)
    bias = np.asarray(bias, np.float32)
    gamma = np.asarray(gamma, np.float32)
    beta = np.asarray(beta, np.float32)

    in_maps, flags = _host_prep(
        x, edge_index, edge_attr, Wl, bl, Wr, br, We, att, bias, gamma, beta
    )

    if flags not in _CACHE:
        nc = _build_nc(flags)
        nc.finalize()
        _CACHE[flags] = nc
    nc = _CACHE[flags]

    from concourse.bass_utils import run_bass_kernel_spmd

    res = run_bass_kernel_spmd(
        nc, in_maps, list(range(N_CORES)), trace=TRACE
    )
    LAST_RESULTS = res
    LAST_EXEC_TIME_NS = res.exec_time_ns
    out = np.concatenate(
        [
            res.results[c]["out"]
            .reshape(128, TILES, 128)
            .transpose(1, 0, 2)
            .reshape(NPAD, 128)[:NPC]
            for c in range(N_CORES)
        ],
        axis=0,
    )
    return out.astype(np.float32)
